# revision 1
# baseline (speedup 1.0000x reference)
"""Deformable cross-attention (KNN/Shepard) Trainium2 kernel.

Sharding: 16 (batch, head) units over 8 cores -> each core handles one batch
and two heads. Within a core:
  - loc/attn projections via PE with an augmented contraction that folds in
    query_pos, biases, and a constant-one row (lhsT blocks come out ready),
  - KNN scores s = 2*loc.kv - |kv|^2 as K=3 fp32 matmuls into PSUM
    (argmax s == argmin dist^2),
  - top-4 neighbors via the DVE max8 / max_index reductions,
  - neighbor distances recovered from the score values
    (d2 = |loc|^2 - s, no position gather needed),
  - value rows fetched with indirect DMA from per-head DRAM tables
    (one offset per partition per DMA -- hardware restriction),
  - Shepard + attention weighting batched per head into wide strided ops;
    weighted sum via one broadcasted multiply + strided reduction,
  - output projection back on PE; host sums the 4 per-batch partials
    (the W_out head reduction) during unshard.

All PSUM users in prologue+main share one two-slot pool so score tiles can
start as soon as the loc projection chain is done instead of waiting for the
whole prologue (PSUM has exactly 8 banks = 2 score tiles).
"""

import os
import sys

for _p in ("/opt/trn_rl_repo", "/root/.axon_site/_ro/trn_rl_repo"):
    if os.path.isdir(_p) and _p not in sys.path:
        sys.path.insert(0, _p)

import numpy as np

import concourse.bass as bass
import concourse.bacc as bacc
import concourse.mybir as mybir
import concourse.tile as tile
from concourse.bass_utils import run_bass_kernel_spmd
from concourse.masks import make_identity

F32 = mybir.dt.float32
U32 = mybir.dt.uint32

B = 2
NQ = 1024
NKV = 2048
D = 256
H = 8
K = 4
NN = 4
C_ = 32  # head dim
N_CORES = 8
QT = NQ // 128  # 8 query tiles per head
KC = NKV // 512  # 4 kv column chunks (PSUM banks)


def build_nc(skip_weights=False, skip_epilogue=False, skip_main=False):
    nc = bacc.Bacc("TRN2", target_bir_lowering=False, debug=False, num_devices=N_CORES)

    qT = nc.dram_tensor("qT", [D, NQ], F32, kind="ExternalInput")
    qp3 = nc.dram_tensor("qp3", [3, NQ], F32, kind="ExternalInput")
    kvT = nc.dram_tensor("kvT", [D, NKV], F32, kind="ExternalInput")
    kvp2 = nc.dram_tensor("kvp2", [2, NKV], F32, kind="ExternalInput")
    wloc = nc.dram_tensor("wloc", [D + 3, 32], F32, kind="ExternalInput")
    wv = nc.dram_tensor("wv", [D, 2 * C_], F32, kind="ExternalInput")
    wout = nc.dram_tensor("wout", [2, C_ + 1, D], F32, kind="ExternalInput")
    spow = nc.dram_tensor("spow", [1, 1], F32, kind="ExternalInput")
    psmat = nc.dram_tensor("psmat", [32, 8], F32, kind="ExternalInput")
    outp = nc.dram_tensor("outp", [NQ, D], F32, kind="ExternalOutput")

    with tile.TileContext(nc) as tc:
        with (
            tc.tile_pool(name="persist", bufs=1) as pp,
            tc.tile_pool(name="dram", bufs=1, space="DRAM") as dp,
        ):
            # ---- persistent SBUF state ----
            qT_sb = [pp.tile([128, NQ], F32, tag=f"qT{i}", name=f"qT{i}") for i in range(2)]
            qp3_sb = pp.tile([3, NQ], F32, tag="qp3", name="qp3")
            kvT_sb = [pp.tile([128, NKV], F32, tag=f"kvT{i}", name=f"kvT{i}") for i in range(2)]
            kvp2_sb = pp.tile([2, NKV], F32, tag="kvp2", name="kvp2")
            wloc_sb = [pp.tile([128, 32], F32, tag=f"wl{i}", name=f"wl{i}") for i in range(2)]
            wloc3_sb = pp.tile([3, 32], F32, tag="wl3", name="wl3")
            wv_sb = [pp.tile([128, 2 * C_], F32, tag=f"wv{i}", name=f"wv{i}") for i in range(2)]
            wout_sb = [pp.tile([C_ + 1, D], F32, tag=f"wo{i}", name=f"wo{i}") for i in range(2)]
            kv_aug = pp.tile([3, NKV], F32, tag="kv_aug", name="kv_aug")
            negp = pp.tile([128, 1], F32, tag="negp", name="negp")
            negp_eps = pp.tile([128, 1], F32, tag="negp_eps", name="negp_eps")
            id16 = pp.tile([16, 16], F32, tag="id16", name="id16")
            id128 = pp.tile([128, 128], F32, tag="id128", name="id128")
            # per (head, k) sampling locations, rows = (x, y, 1)
            loc_sb = [pp.tile([3, NQ], F32, tag=f"loc{i}", name=f"loc{i}") for i in range(2 * K)]
            # per qtile: [h0 logits(4) ll(4) | h1 logits(4) ll(4)]
            mpb = pp.tile([128, 16 * QT], F32, tag="mpb", name="mpb")
            psmat_sb = pp.tile([32, 8], F32, tag="psmat", name="psmat_sb")
            attn_w = [pp.tile([128, 4 * QT], F32, tag=f"aw{i}", name=f"aw{i}") for i in range(2)]
            out_all = [pp.tile([128, QT, C_], F32, tag=f"oa{i}", name=f"oa{i}") for i in range(2)]
            tables = [dp.tile([NKV, C_], F32, tag=f"tab{i}", name=f"tab{i}") for i in range(2)]

            for i in range(2):
                nc.sync.dma_start(qT_sb[i][:], qT[128 * i : 128 * (i + 1), :])
                nc.sync.dma_start(kvT_sb[i][:], kvT[128 * i : 128 * (i + 1), :])
                nc.sync.dma_start(wloc_sb[i][:], wloc[128 * i : 128 * (i + 1), :])
                nc.sync.dma_start(wv_sb[i][:], wv[128 * i : 128 * (i + 1), :])
                nc.sync.dma_start(wout_sb[i][:], wout[i, :, :])
            nc.sync.dma_start(qp3_sb[:], qp3[:])
            nc.sync.dma_start(kvp2_sb[:], kvp2[:])
            nc.sync.dma_start(wloc3_sb[:], wloc[D : D + 3, :])
            nc.sync.dma_start(psmat_sb[:], psmat[:])
            make_identity(nc, id16[:])
            make_identity(nc, id128[:])

            with (
                tc.tile_pool(name="psA", bufs=1, space="PSUM") as psA,
                tc.tile_pool(name="sbA", bufs=2) as sbA,
            ):
                # ---- shepard power -> broadcast -(relu(p)+1e-6) ----
                sp_sb = sbA.tile([1, 1], F32, tag="sp", name="sp")
                nc.sync.dma_start(sp_sb[:], spow[:])
                sp_r = sbA.tile([1, 1], F32, tag="sp_r", name="sp_r")
                nc.scalar.activation(sp_r[:], sp_sb[:], mybir.ActivationFunctionType.Relu)
                np1 = sbA.tile([1, 1], F32, tag="np1", name="np1")
                nc.vector.tensor_scalar(
                    np1[:], sp_r[:], 1e-6, -1.0,
                    op0=mybir.AluOpType.add, op1=mybir.AluOpType.mult,
                )
                np_row = sbA.tile([1, 128], F32, tag="np_row", name="np_row")
                nc.vector.tensor_copy(np_row[:], np1[:].to_broadcast([1, 128]))
                one1 = sbA.tile([1, 1], F32, tag="one1", name="one1")
                nc.vector.memset(one1[:], 1.0)
                np_ps = psA.tile([128, 1], F32, tag="sm", name="np_ps", space="PSUM", bufs=2)
                nc.tensor.matmul(np_ps[:], np_row[:], one1[:], start=True, stop=True)
                nc.scalar.copy(negp[:], np_ps[:])
                nc.vector.tensor_scalar_mul(negp_eps[:], negp[:], 1e-6)

                # ---- kv_aug = [2x; 2y; -(x^2+y^2)] ----
                nc.vector.tensor_scalar_mul(kv_aug[0:2, :], kvp2_sb[:], 2.0)
                kv_sq = sbA.tile([2, NKV], F32, tag="kv_sq", name="kv_sq")
                nc.scalar.activation(kv_sq[:], kvp2_sb[:], mybir.ActivationFunctionType.Square)
                neg2 = sbA.tile([2, 1], F32, tag="neg2", name="neg2")
                nc.vector.memset(neg2[:], -1.0)
                for c in range(KC):
                    kk_ps = psA.tile([1, 512], F32, tag="sm", name="kk_ps", space="PSUM", bufs=2)
                    nc.tensor.matmul(
                        kk_ps[:], neg2[:], kv_sq[:, 512 * c : 512 * (c + 1)],
                        start=True, stop=True,
                    )
                    kkS = sbA.tile([1, 512], F32, tag="kkS", name="kkS")
                    nc.scalar.copy(kkS[:], kk_ps[:])
                    nc.sync.dma_start(kv_aug[2:3, 512 * c : 512 * (c + 1)], kkS[:])

                # ---- loc & attn-logit projection (both heads fused) ----
                miscT = sbA.tile([16, NQ], F32, tag="miscT", name="miscT")
                for ch in range(NQ // 512):
                    sl = slice(512 * ch, 512 * (ch + 1))
                    proj_ps = psA.tile([32, 512], F32, tag="proj", name="proj_ps", space="PSUM", bufs=2)
                    nc.tensor.matmul(proj_ps[:], wloc_sb[0][:], qT_sb[0][:, sl],
                                     start=True, stop=False)
                    nc.tensor.matmul(proj_ps[:], wloc_sb[1][:], qT_sb[1][:, sl],
                                     start=False, stop=False)
                    nc.tensor.matmul(proj_ps[:], wloc3_sb[:], qp3_sb[:, sl],
                                     start=False, stop=True)
                    projS = sbA.tile([32, 512], F32, tag="projS", name="projS")
                    nc.scalar.copy(projS[:], proj_ps[:])
                    for i in range(2 * K):
                        nc.sync.dma_start(loc_sb[i][:, sl], projS[3 * i : 3 * i + 3, :])
                    for h in range(2):
                        nc.sync.dma_start(miscT[8 * h : 8 * h + 4, sl],
                                          projS[24 + 4 * h : 24 + 4 * h + 4, :])
                    # |loc|^2 rows for all (h, k) via one pair-sum matmul
                    sqS = sbA.tile([32, 512], F32, tag="sqS", name="sqS")
                    nc.scalar.activation(sqS[:], projS[:], mybir.ActivationFunctionType.Square)
                    ll_ps = psA.tile([8, 512], F32, tag="sm", name="ll_ps", space="PSUM", bufs=2)
                    nc.tensor.matmul(ll_ps[:], psmat_sb[:], sqS[:], start=True, stop=True)
                    llS = sbA.tile([8, 512], F32, tag="llS", name="llS")
                    nc.scalar.copy(llS[:], ll_ps[:])
                    for h in range(2):
                        nc.sync.dma_start(miscT[8 * h + 4 : 8 * h + 8, sl],
                                          llS[4 * h : 4 * h + 4, :])

                # value-projection: results staged in SBUF, two batched table
                # writes; iterations emitted interleaved with the first score
                # tiles (below) so they share PSUM slots with the main loop.
                vp_all = sbA.tile([128, NKV // 128, 2 * C_], F32, tag="vp_all", name="vp_all")

                def emit_value_proj(t):
                    vp_ps = psA.tile([128, 2 * C_], F32, tag="vp", name="vp_ps", space="PSUM", bufs=2)
                    for i in range(2):
                        nc.tensor.matmul(
                            vp_ps[:],
                            kvT_sb[i][:, 128 * t : 128 * (t + 1)],
                            wv_sb[i][:],
                            start=(i == 0), stop=(i == 1),
                        )
                    nc.scalar.copy(vp_all[:, t, :], vp_ps[:])

                def emit_table_writes():
                    for h in range(2):
                        nc.sync.dma_start(
                            tables[h][:].rearrange("(t p) c -> p t c", p=128),
                            vp_all[:, :, C_ * h : C_ * (h + 1)],
                        )

                def emit_transposes(qts):
                    # transpose [16, NQ] -> point-major [128, 16] per qtile
                    for qt in qts:
                        mp_ps = psA.tile([128, 16], F32, tag="sm", name="mp_ps", space="PSUM", bufs=2)
                        nc.tensor.transpose(
                            mp_ps[:], miscT[:, 128 * qt : 128 * (qt + 1)], id16[:]
                        )
                        nc.scalar.copy(mpb[:, 16 * qt : 16 * (qt + 1)], mp_ps[:])

                def emit_softmax():
                    # attention softmax over K, batched per head
                    for h in range(2):
                        lg = mpb[:].rearrange("p (q e) -> p q e", e=16)[:, :, 8 * h : 8 * h + 4]
                        ae = sbA.tile([128, QT, 4], F32, tag="ae", name="ae")
                        nc.scalar.activation(ae[:], lg, mybir.ActivationFunctionType.Exp)
                        asum = sbA.tile([128, QT], F32, tag="asum", name="asum")
                        nc.vector.tensor_reduce(out=asum[:], in_=ae[:],
                                                axis=mybir.AxisListType.X,
                                                op=mybir.AluOpType.add)
                        arec = sbA.tile([128, QT], F32, tag="arec", name="arec")
                        nc.vector.reciprocal(arec[:], asum[:])
                        nc.vector.tensor_tensor(
                            out=attn_w[h][:].rearrange("p (q k) -> p q k", k=4),
                            in0=ae[:], in1=arec[:].to_broadcast([128, QT, 4]),
                            op=mybir.AluOpType.mult,
                        )

                for t in range(NKV // 128):
                    emit_value_proj(t)
                emit_table_writes()
                emit_transposes(range(QT))
                emit_softmax()

            # ================= main loop: scores + KNN + weights =================
            with (
                tc.tile_pool(name="ps", bufs=2, space="PSUM") as ps,
                tc.tile_pool(name="sbB", bufs=2) as sbB,
            ):

                def emit_weights_half(h, v8a, idxa, q0, qn):
                    qs = slice(q0, q0 + qn)
                    v4 = v8a[:].rearrange("p q (k j) -> p q k j", j=8)[:, qs, :, 0:NN]
                    ll = (
                        mpb[:]
                        .rearrange("p (q e) -> p q e", e=16)[:, qs, 8 * h + 4 : 8 * h + 8]
                        .to_broadcast([128, qn, K, NN])
                    )
                    d2 = sbB.tile([128, qn, K, NN], F32, tag="d2", name="d2", bufs=3)
                    nc.vector.tensor_tensor(out=d2[:], in0=ll, in1=v4,
                                            op=mybir.AluOpType.subtract)
                    nc.gpsimd.tensor_scalar_max(d2[:], d2[:], 0.0)
                    dist = sbB.tile([128, qn, K, NN], F32, tag="dist", name="dist", bufs=3)
                    nc.scalar.activation(dist[:], d2[:], mybir.ActivationFunctionType.Sqrt)
                    ew = sbB.tile([128, qn, K, NN], F32, tag="ew", name="ew", bufs=3)
                    nc.scalar.activation(ew[:], dist[:], mybir.ActivationFunctionType.Exp,
                                         bias=negp_eps[:], scale=negp[:])
                    ssum = sbB.tile([128, qn, K], F32, tag="ssum", name="ssum", bufs=3)
                    nc.vector.tensor_reduce(out=ssum[:], in_=ew[:],
                                            axis=mybir.AxisListType.X,
                                            op=mybir.AluOpType.add)
                    rr = sbB.tile([128, qn, K], F32, tag="rr", name="rr", bufs=3)
                    nc.vector.reciprocal(rr[:], ssum[:])
                    ar = sbB.tile([128, qn, K], F32, tag="ar", name="ar", bufs=3)
                    nc.vector.tensor_mul(
                        ar[:],
                        attn_w[h][:].rearrange("p (q k) -> p q k", k=4)[:, qs, :],
                        rr[:],
                    )
                    ww = sbB.tile([128, qn, K, NN], F32, tag="ww", name="ww", bufs=3)
                    nc.vector.tensor_tensor(out=ww[:], in0=ew[:],
                                            in1=ar[:].to_broadcast([128, qn, K, NN]),
                                            op=mybir.AluOpType.mult)
                    vga = sbB.tile([128, qn, K * NN, C_], F32, tag="vga", name="vga", bufs=2)
                    for qq in range(qn):
                        for k in range(K):
                            for j in range(NN):
                                col = 8 * k + j
                                nc.gpsimd.indirect_dma_start(
                                    out=vga[:, qq, NN * k + j, :], out_offset=None,
                                    in_=tables[h][:],
                                    in_offset=bass.IndirectOffsetOnAxis(
                                        ap=idxa[:, q0 + qq, col : col + 1], axis=0
                                    ),
                                )
                    vgw = sbB.tile([128, qn, K * NN, C_], F32, tag="vgw", name="vgw", bufs=2)
                    nc.gpsimd.tensor_tensor(
                        out=vgw[:], in0=vga[:],
                        in1=ww[:].rearrange("p q k j -> p q (k j)").to_broadcast(
                            [128, qn, K * NN, C_]
                        ),
                        op=mybir.AluOpType.mult,
                    )
                    nc.vector.tensor_reduce(
                        out=out_all[h][:, qs, :],
                        in_=vgw[:].rearrange("p q a c -> p q c a"),
                        axis=mybir.AxisListType.X, op=mybir.AluOpType.add,
                    )

                for h in range(2) if not skip_main else []:
                    v8a = sbB.tile([128, QT, 8 * K], F32, tag="v8a", name="v8a")
                    idxa = sbB.tile([128, QT, 8 * K], U32, tag="idxa", name="idxa")
                    for qt in range(QT):
                        qsl = slice(128 * qt, 128 * (qt + 1))
                        for k in range(K):
                            sc = ps.tile([128, NKV], F32, tag="sc", name="sc", space="PSUM")
                            for c in range(KC):
                                csl = slice(512 * c, 512 * (c + 1))
                                nc.tensor.matmul(
                                    sc[:, csl], loc_sb[K * h + k][:, qsl],
                                    kv_aug[:, csl], start=True, stop=True,
                                )
                            # ACT stages scores to SBUF: frees the PSUM slot for
                            # the PE early and gives DVE the cheaper SBUF reads.
                            scS = sbB.tile([128, NKV], F32, tag="scS", name="scS", bufs=3)
                            nc.scalar.copy(scS[:], sc[:])
                            nc.vector.max(v8a[:, qt, 8 * k : 8 * k + 8], scS[:])
                            nc.vector.max_index(
                                idxa[:, qt, 8 * k : 8 * k + 8],
                                v8a[:, qt, 8 * k : 8 * k + 8], scS[:],
                            )
                    if not skip_weights and qt == QT - 1:
                        if h == 0:
                            emit_weights_half(h, v8a, idxa, 0, QT)
                        else:
                            # split the last head so the epilogue pipelines in
                            emit_weights_half(h, v8a, idxa, 0, QT // 2)
                            emit_weights_half(h, v8a, idxa, QT // 2, QT // 2)
            # ================= epilogue: output projection =================
            with (
                tc.tile_pool(name="psC", bufs=2, space="PSUM") as psC,
                tc.tile_pool(name="sbC", bufs=2) as sbC,
            ):
                for qt in range(QT) if not skip_epilogue else []:
                    o_ps = psC.tile([128, D], F32, tag="o_ps", name="o_ps", space="PSUM")
                    for h in range(2):
                        t_ps = psC.tile([C_, 128], F32, tag="t_ps", name="t_ps", space="PSUM")
                        nc.tensor.transpose(t_ps[:], out_all[h][:, qt, :], id128[:])
                        oT = sbC.tile([C_ + 1, 128], F32, tag="oT", name="oT")
                        nc.scalar.copy(oT[0:C_, :], t_ps[:])
                        nc.vector.memset(oT[C_ : C_ + 1, :], 1.0)
                        nc.tensor.matmul(
                            o_ps[:], oT[:], wout_sb[h][:],
                            start=(h == 0), stop=(h == 1),
                        )
                    o_sb = sbC.tile([128, D], F32, tag="o_sb", name="o_sb")
                    nc.scalar.copy(o_sb[:], o_ps[:])
                    nc.sync.dma_start(outp[128 * qt : 128 * (qt + 1), :], o_sb[:])

    nc.compile()
    return nc


def make_in_maps(inputs):
    """Host-side sharding/layout prep: per-core input dicts."""
    query = np.ascontiguousarray(inputs["query"], dtype=np.float32)
    query_pos = np.ascontiguousarray(inputs["query_pos"], dtype=np.float32)
    key_value = np.ascontiguousarray(inputs["key_value"], dtype=np.float32)
    kv_pos = np.ascontiguousarray(inputs["kv_pos"], dtype=np.float32)
    W_off = np.asarray(inputs["W_off"], dtype=np.float32)
    b_off = np.asarray(inputs["b_off"], dtype=np.float32)
    W_attn = np.asarray(inputs["W_attn"], dtype=np.float32)
    b_attn = np.asarray(inputs["b_attn"], dtype=np.float32)
    W_v = np.asarray(inputs["W_v"], dtype=np.float32)
    b_v = np.asarray(inputs["b_v"], dtype=np.float32)
    W_out = np.asarray(inputs["W_out"], dtype=np.float32)
    b_out = np.asarray(inputs["b_out"], dtype=np.float32)
    sp = np.asarray(inputs["shepard_power"], dtype=np.float32).reshape(1, 1)

    assert np.all(b_v == 0.0), "kernel folds b_v==0; extend wv if nonzero"

    in_maps = []
    for core in range(N_CORES):
        b = core // 4
        h0 = 2 * (core % 4)
        qT = np.ascontiguousarray(query[b].T)
        qp3 = np.concatenate(
            [query_pos[b].T, np.ones((1, NQ), np.float32)], axis=0
        )
        kvT = np.ascontiguousarray(key_value[b].T)
        kvp2 = np.ascontiguousarray(kv_pos[b].T)
        # columns: for i=(j,k) in 0..8: [x_i, y_i, one_i] at 3i..3i+2; attn at 24+4j+k
        wloc = np.zeros((D + 3, 32), np.float32)
        for j in range(2):
            h = h0 + j
            for k in range(4):
                i = 4 * j + k
                wloc[:D, 3 * i] = W_off[:, 8 * h + 2 * k]
                wloc[:D, 3 * i + 1] = W_off[:, 8 * h + 2 * k + 1]
                wloc[D, 3 * i] = 1.0       # + query_pos x
                wloc[D + 1, 3 * i + 1] = 1.0  # + query_pos y
                wloc[D + 2, 3 * i] = b_off[8 * h + 2 * k]
                wloc[D + 2, 3 * i + 1] = b_off[8 * h + 2 * k + 1]
                wloc[D + 2, 3 * i + 2] = 1.0  # constant-one row for the lhsT block
            wloc[:D, 24 + 4 * j : 24 + 4 * j + 4] = W_attn[:, 4 * h : 4 * h + 4]
            wloc[D + 2, 24 + 4 * j : 24 + 4 * j + 4] = b_attn[4 * h : 4 * h + 4]
        psmat = np.zeros((32, 8), np.float32)
        for i in range(8):
            psmat[3 * i, i] = 1.0
            psmat[3 * i + 1, i] = 1.0
        wv = np.concatenate(
            [W_v[:, C_ * (h0 + j) : C_ * (h0 + j + 1)] for j in range(2)], axis=1
        )
        wout = np.zeros((2, C_ + 1, D), np.float32)
        for j in range(2):
            h = h0 + j
            wout[j, :C_, :] = W_out[C_ * h : C_ * (h + 1), :]
        wout[0, C_, :] = b_out / 4.0  # bias split across the 4 cores of a batch
        in_maps.append(
            {
                "qT": qT, "qp3": qp3, "kvT": kvT, "kvp2": kvp2,
                "wloc": wloc, "wv": np.ascontiguousarray(wv),
                "wout": wout, "spow": sp, "psmat": psmat,
            }
        )
    return in_maps


_NC_CACHE = {}


def _get_nc():
    if "nc" not in _NC_CACHE:
        _NC_CACHE["nc"] = build_nc()
    return _NC_CACHE["nc"]


def run(inputs, trace=False):
    nc = _get_nc()
    in_maps = make_in_maps(inputs)
    res = run_bass_kernel_spmd(nc, in_maps, core_ids=list(range(N_CORES)), trace=trace)
    out = np.zeros((B, NQ, D), np.float32)
    for core in range(N_CORES):
        out[core // 4] += res.results[core]["outp"]
    return out, res


def kernel(**inputs):
    out, _ = run(inputs, trace=False)
    return out

